# revision 9
# baseline (speedup 1.0000x reference)
"""Multi-head self-attention (b=4, L=2048, d=512, h=8) on 8 trn2 cores.

Sharding: data-parallel over batch (4) x tensor-parallel over heads (2 groups
of 4).  Core c handles batch c//2, heads [4*(c%2), 4*(c%2)+4).  Each core
returns a partial output (row-parallel Wo); the host sums the two partials per
batch and adds bo.

v2 changes over the 176.6us baseline:
  - scores run as fp8e4m3 DoubleRow matmuls (0.5 cyc/row): Q^T/K^T are kept
    in a "fold" layout [128p = 4 heads x 32, 2 fold-slots, L] produced by
    splitting each projection into two matmuls (A = features 0..31 of each
    head, B = 32..63, via host-permuted weight columns) whose psum tiles
    evacuate lane-aligned into fold slots 0/1 as fp8.  Head h sits at
    partitions [32h, 32h+32); blocks pair heads (pr, pr+2) so the two score
    matmuls land on PE row-tiles (0|32, +64) = T0/T8 and run concurrently.
  - exp is load-balanced: most tiles on ACT (exp), ~2 tiles/block on DVE via
    Schraudolph fast-exp: Wq/bq are pre-scaled by A16 = 2^23/ln2/2^16 (and
    the attention 1/8), so scores psum = A16*score; DVE does a single
    tensor_scalar(+B16) with int16 output whose bit pattern IS bf16 exp.
    ACT tiles use activation(Exp, scale=1/A16).
  - all inputs come in p-major host-folded layouts (per-partition contiguous
    KBs) so DMA descriptors are fat; critical inputs (wk, wq, x chunk 0)
    are prioritized and the warmup is shortened to match.
  - V + ones column -> PV gives O un-normalized plus the softmax denominator
    in row 64 (unchanged from baseline), PV/normalize/out-proj unchanged.
"""

import numpy as np
import ml_dtypes

import concourse.bass as bass
import concourse.bacc as bacc
import concourse.tile as tile
import concourse.mybir as mybir
from concourse.bass_utils import run_bass_kernel_spmd

F32 = mybir.dt.float32
F16 = mybir.dt.float16
BF16 = mybir.dt.bfloat16
F8E4 = mybir.dt.float8e4
I16 = mybir.dt.int16
DR = mybir.MatmulPerfMode.DoubleRow

B, L, D = 4, 2048, 512
NH, DH = 8, 64
HG = 2                 # head pair blocks per core
GH = NH // HG          # 4 heads per group
EG = GH * DH           # 256 columns per group
P = 128
KT = L // P            # 16 k-tiles
QC = L // 512          # 4 q-chunks of 512
DC = D // P            # 4 d-chunks

# Schraudolph fast-exp constants (int16 variant: bits of bf16 exp)
A16 = float((1 << 23) / np.log(2.0) / 65536.0)      # 184.6650
B16 = float((127.0 * (1 << 23) - 486411.0) / 65536.0)   # 16247.577
SCALE = 1.0 / float(np.sqrt(DH))
AP_SCALE = A16 * SCALE          # folded into Wq/bq on the host
INV_A16 = 1.0 / A16             # ACT exp scale

# DVE-exp tile assignment: set of (n, pr, i) handled by fast-exp on DVE.
# Block (0,0) is excluded (DVE busy with V/K/Q evacuations), the last block
# too (tail normalize runs on DVE).
DVE_TILES = set()
for _n in range(QC):
    for _pr in range(HG):
        if (_n, _pr) == (0, 0) or (_n, _pr) == (QC - 1, HG - 1):
            continue
        _is = (7, 12) if (_n, _pr) == (0, 1) else (2, 7, 12)
        DVE_TILES.update((_n, _pr, _i) for _i in _is)

_ts = bass.ts


def _head(pr, j):
    # fold-head of slot j in block pr: pairs (0,2) and (1,3) -> T0/T8 tiles
    return pr + 2 * j


def _body(tc):
    nc = tc.nc
    # p-major host layouts: per-partition contiguous
    xTf = nc.dram_tensor("xTf", [P, QC, DC, 512], BF16, kind="ExternalInput")
    wqf = nc.dram_tensor("wqf", [P, DC, EG], BF16, kind="ExternalInput")
    wkf = nc.dram_tensor("wkf", [P, DC, EG], BF16, kind="ExternalInput")
    wvf = nc.dram_tensor("wvf", [P, DC, EG], BF16, kind="ExternalInput")
    wof = nc.dram_tensor("wof", [P, HG, D], BF16, kind="ExternalInput")
    # packed biases: col 0,1 = bq*AP_SCALE (A,B); col 2,3 = bk (A,B)
    bpk = nc.dram_tensor("bpk", [P, 4], F32, kind="ExternalInput")
    outT = nc.dram_tensor("outT", [P, DC, L], F16, kind="ExternalOutput")

    add = mybir.AluOpType.add
    Exp = mybir.ActivationFunctionType.Exp

    with (
        tc.tile_pool(name="const", bufs=1) as const,
        #   tag "s": score tiles [128,2,512] (2 banks) x2  -> 4 banks
        #   tag "o": PV accumulators [65,512] x2           -> 2 banks
        #   tag "u": proj / out-proj psum [128,512] x2     -> 2 banks
        tc.tile_pool(name="ps", bufs=2, space="PSUM") as ps,
        tc.tile_pool(name="ew", bufs=3) as ew,
        tc.tile_pool(name="nw", bufs=3) as nw,
        tc.tile_pool(name="ow", bufs=4) as ow,
    ):
        wq_sb = const.tile([P, DC, EG], BF16)
        wk_sb = const.tile([P, DC, EG], BF16)
        wv_sb = const.tile([P, DC, EG], BF16)
        xT_sb = const.tile([P, DC, L], BF16)
        wo_sb = const.tile([P, HG, D], BF16)
        bias_sb = const.tile([P, 4], F32)

        def x_load(eng, n, cs=slice(0, DC)):
            eng.dma_start(out=xT_sb[:, cs, _ts(n, 512)], in_=xTf[:, n, cs, :])

        # input loads: critical path to first exp is wk, wq, x0
        nc.sync.dma_start(out=wk_sb[:], in_=wkf[:, :, :])
        nc.scalar.dma_start(out=wq_sb[:], in_=wqf[:, :, :])
        nc.gpsimd.dma_start(out=bias_sb[:], in_=bpk[:, :])
        x_load(nc.sync, 0, slice(0, 2))
        x_load(nc.scalar, 0, slice(2, 4))
        nc.sync.dma_start(out=wv_sb[:, 0:2, :], in_=wvf[:, 0:2, :])
        nc.scalar.dma_start(out=wv_sb[:, 2:4, :], in_=wvf[:, 2:4, :])
        x_load(nc.sync, 1)
        x_load(nc.scalar, 2)
        x_load(nc.sync, 3)
        nc.scalar.dma_start(out=wo_sb[:], in_=wof[:, :, :])

        bq_sb = bias_sb[:, 0:2]     # cols: A, B (pre-scaled by AP_SCALE)
        bk_sb = bias_sb[:, 2:4]

        # persistent activations
        # fold layout: partition 32h+p holds head h feature 32*slot + p
        qt_f8 = const.tile([P, 2, L], F8E4)      # Q fold (A16/8-prescaled)
        kt_f8 = const.tile([P, 2, L], F8E4)      # K fold
        va_sb = const.tile([P, KT, GH, DH + 1], BF16)  # V + ones col
        on_sb = const.tile([P, HG, L], BF16)     # normalized O^T

        nc.vector.memset(va_sb[:, :, :, DH:DH + 1], 1.0)

        # PE warmup: ramp the HAM clock gate to 2.4 GHz before the first
        # projection; shorter than baseline since inputs land sooner.
        warm = const.tile([P, 512], BF16)
        nc.vector.memset(warm[:], 0.0)
        wps = ps.tile([P, 512], F32, tag="u", name="warm_ps")
        for _ in range(42):
            nc.tensor.matmul(wps[:, 0:96], warm[:, 0:P], warm[:, 0:96],
                             start=True, stop=True)

        # ---- projections: A/B column-split, evacuated into fold slots -----
        proj_ps = {}

        def proj_half(w_sb, dst, b_sb, slot, n, half, q0=0, qn=512,
                      on_act=False):
            """One half (2 of 4 c-chunks) of the [128,qn] projection for
            fold-slot `slot` (A=0/B=1), chunk n, column subrange [q0,q0+qn).
            half==1 runs the last two accumulating matmuls and the bias-add
            evacuation into the fp8 fold tile."""
            key = (id(w_sb), slot, n, q0)
            if half == 0:
                proj_ps[key] = ps.tile([P, qn], F32, tag="u",
                                       name=f"p_ps_{slot}_{n}_{q0}",
                                       padded_shape=[P, 512])
            psq = proj_ps[key]
            es = slice(slot * P, slot * P + P)
            qs = slice(n * 512 + q0, n * 512 + q0 + qn)
            for c in (0, 1) if half == 0 else (2, 3):
                nc.tensor.matmul(
                    psq[:],
                    w_sb[:, c, es],
                    xT_sb[:, c, qs],
                    start=(c == 0), stop=(c == DC - 1),
                )
            if half == 1:
                if on_act:
                    nc.scalar.add(dst[:, slot, qs], psq[:],
                                  b_sb[:, slot:slot + 1])
                else:
                    nc.vector.tensor_scalar(
                        out=dst[:, slot, qs], in0=psq[:],
                        scalar1=b_sb[:, slot:slot + 1], scalar2=None, op0=add,
                    )

        def proj_unit(w_sb, dst, b_sb, slot, n, q0=0, qn=512, on_act=False):
            proj_half(w_sb, dst, b_sb, slot, n, 0, q0, qn)
            proj_half(w_sb, dst, b_sb, slot, n, 1, q0, qn, on_act=on_act)

        def v_proj(i):
            psv = ps.tile([P, EG], F32, tag="u", name=f"v_ps_{i}")
            for c in range(DC):
                nc.tensor.matmul(
                    psv[:],
                    xT_sb[:, c, _ts(i, P)],
                    wv_sb[:, c, :],
                    start=(c == 0), stop=(c == DC - 1),
                )
            nc.vector.tensor_copy(
                out=va_sb[:, i, :, 0:DH],
                in_=psv[:].rearrange("p (h d) -> p h d", d=DH),
            )

        def v_proj_half(i, half):
            key = ("v", i)
            if half == 0:
                proj_ps[key] = ps.tile([P, EG], F32, tag="u",
                                       name=f"v_ps_{i}")
            psv = proj_ps[key]
            for c in (0, 1) if half == 0 else (2, 3):
                nc.tensor.matmul(
                    psv[:],
                    xT_sb[:, c, _ts(i, P)],
                    wv_sb[:, c, :],
                    start=(c == 0), stop=(c == DC - 1),
                )
            if half == 1:
                nc.vector.tensor_copy(
                    out=va_sb[:, i, :, 0:DH],
                    in_=psv[:].rearrange("p (h d) -> p h d", d=DH),
                )

        def out_proj_m(n, m, nq=512, qh=0, eng=None, cast_on_act=False):
            eng = eng or nc.sync
            pso = ps.tile([P, 512], F32, tag="u", name=f"o_ps_{m}_{n}_{qh}",
                          padded_shape=[P, 512])
            qs = slice(n * 512 + qh * nq, n * 512 + qh * nq + nq)
            for c in range(HG):
                nc.tensor.matmul(
                    pso[:, 0:nq],
                    wo_sb[:, c, _ts(m, P)],
                    on_sb[:, c, qs],
                    start=(c == 0), stop=(c == HG - 1),
                )
            st = ow.tile([P, 512], F16, tag="o_st", padded_shape=[P, 512],
                         name=f"o_st_{m}_{n}_{qh}")
            if cast_on_act:
                nc.scalar.copy(st[:, 0:nq], pso[:, 0:nq])
            else:
                nc.vector.tensor_copy(st[:, 0:nq], pso[:, 0:nq])
            eng.dma_start(out=outT[:, m, qs], in_=st[:, 0:nq])

        # Upfront: block (0,0) tile 0 needs K fold slots A+B of k-range
        # [0,128) and Q fold slots A+B of q-chunk 0.  The K remainder of
        # chunk 0 (cols 128..512, needed from tile 1) is injected at i=0.
        proj_unit(wk_sb, kt_f8, bk_sb, 0, 0, q0=0, qn=128, on_act=True)
        proj_unit(wk_sb, kt_f8, bk_sb, 1, 0, q0=0, qn=128, on_act=True)
        proj_unit(wq_sb, qt_f8, bq_sb, 0, 0)
        proj_unit(wq_sb, qt_f8, bq_sb, 1, 0)

        def inject(n, pr, i):
            """Filler work (projections, previous chunk's output projection)
            emitted AFTER tile i's attention ops so the FIFO PE queue serves
            scores/PV first."""
            if n == 0 and pr == 0:
                # K chunk-0 remainder, then chunks 1-3 (slots A+B); tile i
                # reads k-chunk i//4, so chunk c must land by tile 4c.
                if i == 0:
                    proj_unit(wk_sb, kt_f8, bk_sb, 0, 0, q0=128, qn=384)
                    proj_unit(wk_sb, kt_f8, bk_sb, 1, 0, q0=128, qn=384,
                              on_act=True)
                elif i == 1:
                    v_proj(0)
                elif i < 15:
                    v_proj(i - 1)
                if i == 2:
                    proj_unit(wk_sb, kt_f8, bk_sb, 0, 1)
                if i == 3:
                    proj_unit(wk_sb, kt_f8, bk_sb, 1, 1)
                if i in (5, 6):
                    proj_half(wk_sb, kt_f8, bk_sb, 0, 2, i - 5)
                if i == 7:
                    proj_unit(wk_sb, kt_f8, bk_sb, 1, 2)
                if i in (9, 10):
                    proj_half(wk_sb, kt_f8, bk_sb, 0, 3, i - 9)
                if i == 11:
                    proj_unit(wk_sb, kt_f8, bk_sb, 1, 3)
                if i == 15:
                    v_proj(14)
            elif n == 0 and pr == 1:
                if i == 0:
                    v_proj(15)
                if i in (1, 2):
                    proj_half(wq_sb, qt_f8, bq_sb, 0, 1, i - 1)
                if i in (5, 6):
                    proj_half(wq_sb, qt_f8, bq_sb, 1, 1, i - 5)
            else:
                # Q chunk n+1: slot A during (n,0), slot B during (n,1)
                if i in (5, 6) and n + 1 < QC:
                    proj_half(wq_sb, qt_f8, bq_sb, pr, n + 1, i - 5)
                if pr == 0 and i in (8, 10, 12, 14):
                    out_proj_m(n - 1, (i - 8) // 2)

        def norm_evac(n, pr, o_t, j, nq=512, qh=0):
            ocp = nw.tile([DH + 1, 512], F32, tag="ocp", bufs=3,
                          padded_shape=[DH + 1, 512],
                          name=f"ocp_{n}_{pr}_{j}_{qh}")
            nc.vector.tensor_copy(ocp[:, 0:nq],
                                  o_t[j][:, qh * nq:qh * nq + nq])
            return ocp

        def norm_recip(n, pr, ocp, j, nq=512, qh=0):
            off = 0 if ocp.shape[-1] == 512 and nq == 512 else qh * nq
            dsb = nw.tile([1, 512], F32, tag="dsb", padded_shape=[1, 512],
                          name=f"dsb_{n}_{pr}_{j}_{qh}")
            nc.vector.tensor_copy(dsb[:, 0:nq], ocp[DH:DH + 1, off:off + nq])
            r = nw.tile([1, 512], F32, tag="r", padded_shape=[1, 512],
                        name=f"r_{n}_{pr}_{j}_{qh}")
            nc.vector.reciprocal_approx_fast(r[:, 0:nq], dsb[:, 0:nq])
            rb = nw.tile([DH, 512], F32, tag="rb", padded_shape=[DH, 512],
                         name=f"rb_{n}_{pr}_{j}_{qh}")
            nc.gpsimd.partition_broadcast(rb[:, 0:nq], r[:, 0:nq])
            return rb

        def norm_mul(n, pr, ocp, rb, j, nq=512, qh=0):
            qs = slice(n * 512 + qh * nq, n * 512 + qh * nq + nq)
            off = 0 if ocp.shape[-1] == 512 and nq == 512 else qh * nq
            nc.vector.tensor_mul(out=on_sb[_ts(j, DH), pr, qs],
                                 in0=ocp[0:DH, off:off + nq],
                                 in1=rb[:, 0:nq])

        def norm_finish(n, pr, ocp, j, nq=512, qh=0):
            rb = norm_recip(n, pr, ocp, j, nq, qh)
            norm_mul(n, pr, ocp, rb, j, nq, qh)

        # ---- attention + interleaved output projection --------------------
        pending = []

        def flush_one():
            pn, ppr, pi, po_t, pet = pending.pop(0)
            for j in range(2):
                nc.tensor.matmul(
                    po_t[j][:],
                    va_sb[:, pi, _head(ppr, j), :],
                    pet[:, j, :],
                    start=(pi == 0), stop=(pi == KT - 1),
                )
            if pi == KT - 1:
                last = (pn == QC - 1) and (ppr == HG - 1)
                if not last:
                    ocps = [norm_evac(pn, ppr, po_t, j) for j in range(2)]
                    for j in range(2):
                        norm_finish(pn, ppr, ocps[j], j)
                else:
                    # tail: normalize straight out of PSUM in q-halves
                    rbs = {}
                    for qh in range(2):
                        for j in range(2):
                            rbs[qh, j] = norm_recip(pn, ppr, po_t[j], j,
                                                    nq=256, qh=qh)
                    for qh in range(2):
                        for j in range(2):
                            norm_mul(pn, ppr, po_t[j], rbs[qh, j], j,
                                     nq=256, qh=qh)
                        for m in range(4):
                            out_proj_m(pn, m, nq=256, qh=qh,
                                       eng=(nc.sync, nc.scalar)[m % 2],
                                       cast_on_act=(m % 2 == 1))

        for n in range(QC):          # q chunk of 512
            for pr in range(HG):     # head pair (fold-heads pr, pr+2)
                o_t = [ps.tile([DH + 1, 512], F32, tag="o", bufs=2,
                               name=f"o_{pr}_{n}_{j}")
                       for j in range(2)]
                last_blk = (n == QC - 1) and (pr == HG - 1)
                for i in range(KT):
                    s = ps.tile([P, 2, 512], F32, tag="s", bufs=2,
                                name=f"s_{pr}_{n}_{i}")
                    for j in range(2):
                        h = _head(pr, j)
                        nc.tensor.matmul(
                            s[:, j, :],
                            kt_f8[_ts(h, 32), :, _ts(i, P)],
                            qt_f8[_ts(h, 32), :, _ts(n, 512)],
                            start=True, stop=True, perf_mode=DR,
                            tile_position=(32 * h, 0),
                        )
                    et = ew.tile([P, 2, 512], BF16, tag="et", bufs=10)
                    if (n, pr, i) in DVE_TILES:
                        # Schraudolph fast-exp: int16(s + B16) bits are bf16
                        nc.vector.tensor_scalar(
                            out=et[:].bitcast(I16), in0=s[:],
                            scalar1=B16, scalar2=None, op0=add)
                    else:
                        nc.scalar.activation(et[:], s[:], Exp, scale=INV_A16)
                    lim = 2 if (last_blk and i >= 13) else 4
                    while len(pending) >= lim:
                        flush_one()
                    pending.append((n, pr, i, o_t, et))
                    inject(n, pr, i)
        while pending:
            flush_one()


_CACHE = {}


def _get_nc():
    if "nc" not in _CACHE:
        nc = bacc.Bacc(None, target_bir_lowering=False)
        with tile.TileContext(nc) as tc:
            _body(tc)
        nc.finalize()
        _CACHE["nc"] = nc
    return _CACHE["nc"]


# fold permutation: fold partition 32h+p, slot i  <->  feature h*64 + 32i + p
_FPERM_A = np.array([(r // 32) * 64 + (r % 32) for r in range(P)])
_FPERM_B = _FPERM_A + 32
_FPERM = np.concatenate([_FPERM_A, _FPERM_B])          # wq/wk column order
# wo partition p, block pr -> feature (pr + 2*(p//64))*64 + p%64
_WOPERM = np.array([[(pr + 2 * (p // 64)) * 64 + (p % 64) for pr in range(HG)]
                    for p in range(P)])


def make_in_maps(x, Wq, bq, Wk, bk, Wv, bv, Wo):
    bf = ml_dtypes.bfloat16
    in_maps = []
    x = np.asarray(x, np.float32)
    Wq = np.asarray(Wq, np.float32)
    Wk = np.asarray(Wk, np.float32)
    Wv = np.asarray(Wv, np.float32)
    Wo = np.asarray(Wo, np.float32)
    bq = np.asarray(bq, np.float32)
    bk = np.asarray(bk, np.float32)
    for c in range(8):
        b, g = c // 2, c % 2
        es = slice(g * EG, (g + 1) * EG)
        # x p-major fold: xTf[p, n, c, q'] = x[b].T[c*128+p, n*512+q']
        xT = np.ascontiguousarray(x[b].T)                       # [D, L]
        xTf = xT.reshape(DC, P, QC, 512).transpose(1, 2, 0, 3)  # [P,QC,DC,512]
        # weights p-major: w[p, c, e] = W[es][perm].T[c*128+p, e]
        def wfold(W, perm=None, scale=1.0):
            Wt = W[es, :] * scale
            if perm is not None:
                Wt = Wt[perm, :]
            return np.ascontiguousarray(
                Wt.T.reshape(DC, P, -1).transpose(1, 0, 2))     # [P, DC, e]
        wqf = wfold(Wq, _FPERM, AP_SCALE)
        wkf = wfold(Wk, _FPERM)
        wvf = wfold(Wv)
        # wo: wof[p, pr, d] = Wo[d, es][:, woperm[p, pr]]
        Wog = Wo[:, es]                                         # [D, 256]
        wof = Wog[:, _WOPERM].transpose(1, 2, 0)                # [P, HG, D]
        bpk = np.zeros((P, 4), np.float32)
        bpk[:, 0] = (bq[es] * AP_SCALE)[_FPERM_A]
        bpk[:, 1] = (bq[es] * AP_SCALE)[_FPERM_B]
        bpk[:, 2] = bk[es][_FPERM_A]
        bpk[:, 3] = bk[es][_FPERM_B]
        in_maps.append({
            "xTf": np.ascontiguousarray(xTf).astype(bf),
            "wqf": np.ascontiguousarray(wqf).astype(bf),
            "wkf": np.ascontiguousarray(wkf).astype(bf),
            "wvf": np.ascontiguousarray(wvf).astype(bf),
            "wof": np.ascontiguousarray(wof).astype(bf),
            "bpk": bpk,
        })
    return in_maps


def gather_out(results, bo, bv, Wo):
    # device partials exclude the V bias: (O/d) @ Wo.  bv passes through the
    # attention untouched (softmax rows sum to 1): host adds bo + bv @ Wo.T.
    const = (np.asarray(bo, np.float64)
             + np.asarray(bv, np.float64) @ np.asarray(Wo, np.float64).T
             ).astype(np.float32)
    out = np.empty((B, L, D), np.float32)
    for b in range(B):
        # outT [P, DC, L] p-major -> [D, L] -> [L, D]
        o0 = results[2 * b]["outT"].astype(np.float32)
        o1 = results[2 * b + 1]["outT"].astype(np.float32)
        full = (o0 + o1).transpose(1, 0, 2).reshape(D, L)
        out[b] = full.T + const[None, :]
    return out


def kernel(x, Wq, bq, Wk, bk, Wv, bv, Wo, bo, **kwargs):
    nc = _get_nc()
    in_maps = make_in_maps(x, Wq, bq, Wk, bk, Wv, bv, Wo)
    res = run_bass_kernel_spmd(nc, in_maps, list(range(8)))
    return gather_out(res.results, bo, bv, Wo)


# revision 10
# speedup vs baseline: 1.1796x; 1.1796x over previous
"""Multi-head self-attention (b=4, L=2048, d=512, h=8) on 8 trn2 cores.

Sharding: data-parallel over batch (4) x tensor-parallel over heads (2 groups
of 4).  Core c handles batch c//2, heads [4*(c%2), 4*(c%2)+4).  Each core
returns a partial output (row-parallel Wo); the host sums the two partials per
batch and adds bo.

Device-side layout (all "transposed", so no on-device transposes are needed):
  xT   [512 d, 2048 q]   (host passes x[b].T, bf16, p-major folded)
  Q^T  [256 hd, 2048 q]  = WqT.T @ xT, pre-scaled by A16*SCALE (see below)
  K^T  same (unscaled)
  V    [2048 k, 256 hd]  (lhsT = xT chunks)  + ones column per head -> V_aug
  S^T  [128 k-tile, 512 q] = A16 * score; PE 64-row mode, two heads of a
       pair run on independent half-arrays (T0 / T8)
  E^T: most tiles on ACT as exp(S^T / A16); ~2-3 tiles per block on DVE via
       Schraudolph fast-exp: one tensor_scalar(+B16) with int16 output whose
       bit pattern IS bf16 exp (Wq/bq carry the A16*SCALE pre-scale).
  O^T_aug [65, 512q] += V_aug_h.T @ E^T_h  (denominator in row 64)
  normalize: O = O[0:64] * broadcast(1/d); host adds bo + bv @ Wo.T
  outT [128, 4, 2048] p-major fp16 partial, host sums the two head groups

Scheduling follows the 176.6us baseline (ACT ~98% busy in steady state, PE
saturated underneath), plus:
  - host-folded p-major DMA layouts (per-partition contiguous KBs -> fat
    descriptors), critical inputs (wk, wq, x0) first; shorter PE warmup
  - DVE fast-exp on ~18 tiles trims the ACT-bound critical path
  - the tail flushes the PV queue early (depth 2 over the last tiles) so
    less work serializes after the final exp
"""

import numpy as np
import ml_dtypes

import concourse.bass as bass
import concourse.bacc as bacc
import concourse.tile as tile
import concourse.mybir as mybir
from concourse.bass_utils import run_bass_kernel_spmd

F32 = mybir.dt.float32
F16 = mybir.dt.float16
BF16 = mybir.dt.bfloat16
I16 = mybir.dt.int16

B, L, D = 4, 2048, 512
NH, DH = 8, 64
HG = 2                 # head groups (tensor parallel)
GH = NH // HG          # 4 heads per group
EG = GH * DH           # 256 columns per group
P = 128
KT = L // P            # 16 k-tiles
QC = L // 512          # 4 q-chunks of 512
DC = D // P            # 4 d-chunks

# Schraudolph fast-exp constants (int16 variant: bits of bf16 exp)
A16 = float((1 << 23) / np.log(2.0) / 65536.0)          # 184.6650
B16 = float((127.0 * (1 << 23) - 486411.0) / 65536.0)   # 16247.577
SCALE = 1.0 / float(np.sqrt(DH))
AP_SCALE = A16 * SCALE          # folded into Wq/bq on the host
INV_A16 = 1.0 / A16             # ACT exp scale

# DVE-exp tile assignment: (n, pr, i) handled by fast-exp on DVE.  Block
# (0,0) is excluded (DVE busy with V/K/Q evacuations), the last block too
# (tail normalize runs on DVE); (0,1) still carries K/Q evacs -> 2 tiles.
DVE_TILES = set()
for _n in range(QC):
    for _pr in range(HG):
        if (_n, _pr) == (0, 0) or (_n, _pr) == (QC - 1, HG - 1):
            continue
        _is = (7, 12) if (_n, _pr) == (0, 1) else (2, 7, 12)
        DVE_TILES.update((_n, _pr, _i) for _i in _is)

_ts = bass.ts


def _body(tc):
    nc = tc.nc
    # p-major host layouts: per-partition contiguous
    xTf = nc.dram_tensor("xTf", [P, QC, DC, 512], BF16, kind="ExternalInput")
    wqf = nc.dram_tensor("wqf", [P, DC, EG], BF16, kind="ExternalInput")
    wkf = nc.dram_tensor("wkf", [P, DC, EG], BF16, kind="ExternalInput")
    wvf = nc.dram_tensor("wvf", [P, DC, EG], BF16, kind="ExternalInput")
    wof = nc.dram_tensor("wof", [P, HG, D], BF16, kind="ExternalInput")
    # packed biases: col 0,1 = bq*AP_SCALE (t0,t1); col 2,3 = bk
    bpk = nc.dram_tensor("bpk", [P, 4], F32, kind="ExternalInput")
    outT = nc.dram_tensor("outT", [P, DC, L], F16, kind="ExternalOutput")

    add = mybir.AluOpType.add
    Exp = mybir.ActivationFunctionType.Exp

    with (
        tc.tile_pool(name="const", bufs=1) as const,
        # one PSUM pool for the whole kernel so the phases can overlap:
        #   tag "s": score tiles [128,2,512] (2 banks) x2  -> 4 banks
        #   tag "o": PV accumulators [65,512] x2           -> 2 banks
        #   tag "u": proj / out-proj psum [128,512] x2     -> 2 banks
        tc.tile_pool(name="ps", bufs=2, space="PSUM") as ps,
        tc.tile_pool(name="ew", bufs=3) as ew,
        tc.tile_pool(name="nw", bufs=3) as nw,
        tc.tile_pool(name="ow", bufs=4) as ow,
    ):
        wq_sb = const.tile([P, DC, EG], BF16)
        wk_sb = const.tile([P, DC, EG], BF16)
        wv_sb = const.tile([P, DC, EG], BF16)
        xT_sb = const.tile([P, DC, L], BF16)
        wo_sb = const.tile([P, HG, D], BF16)
        bias_sb = const.tile([P, 4], F32)

        def w_load(eng, w_sb, w_dr, t):
            eng.dma_start(out=w_sb[:, :, _ts(t, P)], in_=w_dr[:, :, _ts(t, P)])

        def x_load(eng, n, cs=slice(0, DC)):
            eng.dma_start(out=xT_sb[:, cs, _ts(n, 512)], in_=xTf[:, n, cs, :])

        # critical path to the first exp: wk(t0), x0, wq(t0), biases
        w_load(nc.sync, wk_sb, wkf, 0)
        w_load(nc.scalar, wq_sb, wqf, 0)
        nc.gpsimd.dma_start(out=bias_sb[:], in_=bpk[:, :])
        x_load(nc.sync, 0, slice(0, 1))
        x_load(nc.scalar, 0, slice(1, 2))
        x_load(nc.sync, 0, slice(2, 3))
        x_load(nc.scalar, 0, slice(3, 4))
        nc.sync.dma_start(out=wv_sb[:, 0:2, :], in_=wvf[:, 0:2, :])
        nc.scalar.dma_start(out=wv_sb[:, 2:4, :], in_=wvf[:, 2:4, :])
        x_load(nc.sync, 1)
        x_load(nc.scalar, 2)
        w_load(nc.sync, wk_sb, wkf, 1)
        w_load(nc.scalar, wq_sb, wqf, 1)
        x_load(nc.sync, 3)
        nc.scalar.dma_start(out=wo_sb[:], in_=wof[:, :, :])

        bq_sb = bias_sb[:, 0:2]
        bk_sb = bias_sb[:, 2:4]

        # persistent activations
        qt_sb = const.tile([P, HG, L], BF16)     # Q^T (A16/8-prescaled)
        kt_sb = const.tile([P, HG, L], BF16)     # K^T
        va_sb = const.tile([P, KT, GH, DH + 1], BF16)  # V + ones col
        on_sb = const.tile([P, HG, L], BF16)     # normalized O^T

        nc.vector.memset(va_sb[:, :, :, DH:DH + 1], 1.0)

        # PE warmup: ramp the HAM clock gate to 2.4 GHz before the first
        # projection; shorter than before since inputs land sooner with the
        # p-major DMA layouts.
        warm = const.tile([P, 512], BF16)
        nc.vector.memset(warm[:], 0.0)
        wps = ps.tile([P, 512], F32, tag="u", name="warm_ps")
        for _ in range(46):
            nc.tensor.matmul(wps[:, 0:96], warm[:, 0:P], warm[:, 0:96],
                             start=True, stop=True)

        # ---- projections (128-row PE mode), emitted just-in-time ----------
        proj_ps = {}

        def proj_half(w_sb, dst, b_sb, t, n, half, on_act=False):
            key = (id(w_sb), t, n)
            if half == 0:
                proj_ps[key] = ps.tile([P, 512], F32, tag="u",
                                       name=f"p_ps_{t}_{n}")
            psq = proj_ps[key]
            for c in (0, 1) if half == 0 else (2, 3):
                nc.tensor.matmul(
                    psq[:],
                    w_sb[:, c, _ts(t, P)],
                    xT_sb[:, c, _ts(n, 512)],
                    start=(c == 0), stop=(c == DC - 1),
                )
            if half == 1:
                if on_act:
                    nc.scalar.add(dst[:, t, _ts(n, 512)], psq[:],
                                  b_sb[:, t:t + 1])
                else:
                    nc.vector.tensor_scalar(
                        out=dst[:, t, _ts(n, 512)], in0=psq[:],
                        scalar1=b_sb[:, t:t + 1], scalar2=None, op0=add,
                    )

        def v_proj(i):
            psv = ps.tile([P, EG], F32, tag="u", name=f"v_ps_{i}")
            for c in range(DC):
                nc.tensor.matmul(
                    psv[:],
                    xT_sb[:, c, _ts(i, P)],
                    wv_sb[:, c, :],
                    start=(c == 0), stop=(c == DC - 1),
                )
            nc.vector.tensor_copy(
                out=va_sb[:, i, :, 0:DH],
                in_=psv[:].rearrange("p (h d) -> p h d", d=DH),
            )

        def v_proj_half(i, half):
            key = ("v", i)
            if half == 0:
                proj_ps[key] = ps.tile([P, EG], F32, tag="u",
                                       name=f"v_ps_{i}")
            psv = proj_ps[key]
            for c in (0, 1) if half == 0 else (2, 3):
                nc.tensor.matmul(
                    psv[:],
                    xT_sb[:, c, _ts(i, P)],
                    wv_sb[:, c, :],
                    start=(c == 0), stop=(c == DC - 1),
                )
            if half == 1:
                nc.vector.tensor_copy(
                    out=va_sb[:, i, :, 0:DH],
                    in_=psv[:].rearrange("p (h d) -> p h d", d=DH),
                )

        def out_proj_m(n, m, nq=512, qh=0, eng=None, cast_on_act=False):
            eng = eng or nc.sync
            pso = ps.tile([P, 512], F32, tag="u", name=f"o_ps_{m}_{n}_{qh}",
                          padded_shape=[P, 512])
            qs = slice(n * 512 + qh * nq, n * 512 + qh * nq + nq)
            for c in range(HG):
                nc.tensor.matmul(
                    pso[:, 0:nq],
                    wo_sb[:, c, _ts(m, P)],
                    on_sb[:, c, qs],
                    start=(c == 0), stop=(c == HG - 1),
                )
            st = ow.tile([P, 512], F16, tag="o_st", padded_shape=[P, 512],
                         name=f"o_st_{m}_{n}_{qh}")
            if cast_on_act:
                nc.scalar.copy(st[:, 0:nq], pso[:, 0:nq])
            else:
                nc.vector.tensor_copy(st[:, 0:nq], pso[:, 0:nq])
            eng.dma_start(out=outT[:, m, qs], in_=st[:, 0:nq])

        # Upfront: only what the first k-tiles of block (0,0) need.
        proj_half(wk_sb, kt_sb, bk_sb, 0, 0, 0)
        proj_half(wq_sb, qt_sb, bq_sb, 0, 0, 0)
        proj_half(wk_sb, kt_sb, bk_sb, 0, 0, 1, on_act=True)
        proj_half(wq_sb, qt_sb, bq_sb, 0, 0, 1)

        def inject(n, pr, i):
            """Emit filler work (projections, previous chunk's output
            projection) AFTER tile i's attention ops so the FIFO PE queue
            serves scores/PV first and the filler soaks up ACT-bound slack."""
            if n == 0 and pr == 0:
                if i == 0:
                    v_proj_half(1, 0)
                    v_proj_half(1, 1)
                elif i == 1:
                    v_proj(2)
                elif i < 15:
                    v_proj(i + 1)
                if i in (2, 3):          # K(t0,n1)
                    proj_half(wk_sb, kt_sb, bk_sb, 0, 1, i - 2)
                if i in (5, 6):          # K(t0,n2)
                    proj_half(wk_sb, kt_sb, bk_sb, 0, 2, i - 5)
                if i in (8, 9):          # K(t0,n3)
                    proj_half(wk_sb, kt_sb, bk_sb, 0, 3, i - 8)
                if i in (11, 12):        # K(t1,n0)
                    proj_half(wk_sb, kt_sb, bk_sb, 1, 0, i - 11)
                if i in (13, 14):
                    proj_half(wq_sb, qt_sb, bq_sb, 1, 0, i - 13)
            elif n == 0 and pr == 1:
                if i in (0, 1):
                    proj_half(wk_sb, kt_sb, bk_sb, 1, 1, i)
                if i in (2, 3):
                    proj_half(wk_sb, kt_sb, bk_sb, 1, 2, i - 2)
                if i in (5, 6):
                    proj_half(wk_sb, kt_sb, bk_sb, 1, 3, i - 5)
                if i in (8, 9):
                    proj_half(wq_sb, qt_sb, bq_sb, 0, 1, i - 8)
            else:
                if i in (5, 6):
                    if pr == 1 and n + 1 < QC:
                        proj_half(wq_sb, qt_sb, bq_sb, 0, n + 1, i - 5)
                    elif pr == 0:
                        proj_half(wq_sb, qt_sb, bq_sb, 1, n, i - 5)
                if pr == 0 and i in (8, 10, 12, 14):
                    out_proj_m(n - 1, (i - 8) // 2)

        def norm_evac(n, pr, o_t, j, nq=512, qh=0):
            ocp = nw.tile([DH + 1, 512], F32, tag="ocp", bufs=3,
                          padded_shape=[DH + 1, 512],
                          name=f"ocp_{n}_{pr}_{j}_{qh}")
            nc.vector.tensor_copy(ocp[:, 0:nq],
                                  o_t[j][:, qh * nq:qh * nq + nq])
            return ocp

        def norm_recip(n, pr, ocp, j, nq=512, qh=0):
            off = 0 if ocp.shape[-1] == 512 and nq == 512 else qh * nq
            dsb = nw.tile([1, 512], F32, tag="dsb", padded_shape=[1, 512],
                          name=f"dsb_{n}_{pr}_{j}_{qh}")
            nc.vector.tensor_copy(dsb[:, 0:nq], ocp[DH:DH + 1, off:off + nq])
            r = nw.tile([1, 512], F32, tag="r", padded_shape=[1, 512],
                        name=f"r_{n}_{pr}_{j}_{qh}")
            nc.vector.reciprocal_approx_fast(r[:, 0:nq], dsb[:, 0:nq])
            rb = nw.tile([DH, 512], F32, tag="rb", padded_shape=[DH, 512],
                         name=f"rb_{n}_{pr}_{j}_{qh}")
            nc.gpsimd.partition_broadcast(rb[:, 0:nq], r[:, 0:nq])
            return rb

        def norm_mul(n, pr, ocp, rb, j, nq=512, qh=0):
            qs = slice(n * 512 + qh * nq, n * 512 + qh * nq + nq)
            off = 0 if ocp.shape[-1] == 512 and nq == 512 else qh * nq
            nc.vector.tensor_mul(out=on_sb[_ts(j, DH), pr, qs],
                                 in0=ocp[0:DH, off:off + nq],
                                 in1=rb[:, 0:nq])

        def norm_finish(n, pr, ocp, j, nq=512, qh=0):
            rb = norm_recip(n, pr, ocp, j, nq, qh)
            norm_mul(n, pr, ocp, rb, j, nq, qh)

        # ---- attention (64-row PE mode) + interleaved output projection ---
        pending = []

        def flush_one():
            pn, ppr, pi, po_t, pet = pending.pop(0)
            for j in range(2):
                nc.tensor.matmul(
                    po_t[j][:],
                    va_sb[:, pi, 2 * ppr + j, :],
                    pet[:, j, :],
                    start=(pi == 0), stop=(pi == KT - 1),
                )
            if pi == KT - 1:
                last = (pn == QC - 1) and (ppr == HG - 1)
                if not last:
                    ocps = [norm_evac(pn, ppr, po_t, j) for j in range(2)]
                    for j in range(2):
                        norm_finish(pn, ppr, ocps[j], j)
                else:
                    # tail: normalize straight out of PSUM, q-halves,
                    # out-DMAs spread across queues, casts split ACT/DVE
                    rbs = {}
                    for qh in range(2):
                        for j in range(2):
                            rbs[qh, j] = norm_recip(pn, ppr, po_t[j], j,
                                                    nq=256, qh=qh)
                    for qh in range(2):
                        for j in range(2):
                            norm_mul(pn, ppr, po_t[j], rbs[qh, j], j,
                                     nq=256, qh=qh)
                        for m in range(4):
                            out_proj_m(pn, m, nq=256, qh=qh,
                                       eng=(nc.sync, nc.scalar)[m % 2],
                                       cast_on_act=(m % 2 == 1))

        for n in range(QC):          # q chunk of 512
            for pr in range(HG):     # head pair (heads 2pr, 2pr+1)
                o_t = [ps.tile([DH + 1, 512], F32, tag="o", bufs=2,
                               name=f"o_{pr}_{n}_{j}")
                       for j in range(2)]
                last_blk = (n == QC - 1) and (pr == HG - 1)
                for i in range(KT):
                    s = ps.tile([P, 2, 512], F32, tag="s", bufs=2,
                                name=f"s_{pr}_{n}_{i}")
                    for j in range(2):
                        nc.tensor.matmul(
                            s[:, j, :],
                            kt_sb[_ts(j, DH), pr, _ts(i, P)],
                            qt_sb[_ts(j, DH), pr, _ts(n, 512)],
                            start=True, stop=True,
                        )
                    et = ew.tile([P, 2, 512], BF16, tag="et", bufs=10)
                    if (n, pr, i) in DVE_TILES:
                        # Schraudolph fast-exp: int16(s + B16) bits = bf16 exp
                        nc.vector.tensor_scalar(
                            out=et[:].bitcast(I16), in0=s[:],
                            scalar1=B16, scalar2=None, op0=add)
                    else:
                        nc.scalar.activation(et[:], s[:], Exp, scale=INV_A16)
                    if n == 0 and pr == 0 and i == 0:
                        v_proj_half(0, 0)
                        v_proj_half(0, 1)
                    lim = 2 if (last_blk and i >= 13) else 4
                    while len(pending) >= lim:
                        flush_one()
                    pending.append((n, pr, i, o_t, et))
                    inject(n, pr, i)
        while pending:
            flush_one()


_CACHE = {}


def _get_nc():
    if "nc" not in _CACHE:
        nc = bacc.Bacc(None, target_bir_lowering=False)
        with tile.TileContext(nc) as tc:
            _body(tc)
        nc.finalize()
        _CACHE["nc"] = nc
    return _CACHE["nc"]


def make_in_maps(x, Wq, bq, Wk, bk, Wv, bv, Wo):
    bf = ml_dtypes.bfloat16
    x = np.asarray(x, np.float32)
    Wq = np.asarray(Wq, np.float32)
    Wk = np.asarray(Wk, np.float32)
    Wv = np.asarray(Wv, np.float32)
    Wo = np.asarray(Wo, np.float32)
    bq = np.asarray(bq, np.float32)
    bk = np.asarray(bk, np.float32)
    in_maps = []
    for c in range(8):
        b, g = c // 2, c % 2
        es = slice(g * EG, (g + 1) * EG)
        xT = np.ascontiguousarray(x[b].T)                       # [D, L]
        xTf = xT.reshape(DC, P, QC, 512).transpose(1, 2, 0, 3)  # [P,QC,DC,512]

        def wfold(W, scale=1.0):
            # [P, DC, e]: wfold[p, c, e] = (W[es]*scale).T[c*128+p, e]
            return np.ascontiguousarray(
                (W[es, :] * scale).T.reshape(DC, P, -1).transpose(1, 0, 2))

        wof = np.ascontiguousarray(
            Wo[:, es].T.reshape(HG, P, D).transpose(1, 0, 2))   # [P, HG, D]
        bpk = np.zeros((P, 4), np.float32)
        bpk[:, 0] = (bq[es] * AP_SCALE)[0:P]
        bpk[:, 1] = (bq[es] * AP_SCALE)[P:2 * P]
        bpk[:, 2] = bk[es][0:P]
        bpk[:, 3] = bk[es][P:2 * P]
        in_maps.append({
            "xTf": np.ascontiguousarray(xTf).astype(bf),
            "wqf": wfold(Wq, AP_SCALE).astype(bf),
            "wkf": wfold(Wk).astype(bf),
            "wvf": wfold(Wv).astype(bf),
            "wof": wof.astype(bf),
            "bpk": bpk,
        })
    return in_maps


def gather_out(results, bo, bv, Wo):
    # device partials exclude the V bias: (O/d) @ Wo.  bv passes through the
    # attention untouched (softmax rows sum to 1): host adds bo + bv @ Wo.T.
    const = (np.asarray(bo, np.float64)
             + np.asarray(bv, np.float64) @ np.asarray(Wo, np.float64).T
             ).astype(np.float32)
    out = np.empty((B, L, D), np.float32)
    for b in range(B):
        o0 = results[2 * b]["outT"].astype(np.float32)
        o1 = results[2 * b + 1]["outT"].astype(np.float32)
        full = (o0 + o1).transpose(1, 0, 2).reshape(D, L)
        out[b] = full.T + const[None, :]
    return out


def kernel(x, Wq, bq, Wk, bk, Wv, bv, Wo, bo, **kwargs):
    nc = _get_nc()
    in_maps = make_in_maps(x, Wq, bq, Wk, bk, Wv, bv, Wo)
    res = run_bass_kernel_spmd(nc, in_maps, list(range(8)))
    return gather_out(res.results, bo, bv, Wo)
